# revision 25
# baseline (speedup 1.0000x reference)
"""GQA attention block (RMSNorm-QK, causal, GQA) on 8 trn2 NeuronCores.

Sequence sharding, zero collectives. Core c handles batch c//4 and two
causally-balanced query chunks (j and 7-j of 8, 256 rows each). Host permutes
the key axis per core: [A | B | c1 | c2 | c3] so queries sit at columns
[0:512] and causality is data (two fixed triangle masks + per-core 0/1
column-kill scalars + exp bias columns); one uniform SPMD program serves all
cores.

All matmul operands are bf16 (fp32 PSUM accumulation). Activations are
feature-major ([feature, token]); V is projected token-major directly
(lhsT = x tile), so no on-device transposes at all. Scores are [k, q];
softmax needs no max subtraction (RMS-normalized q,k bound |s|/sqrt(D)).
Attention is causally trimmed: per head 8 N=512 score tiles (both chunks)
+ 8 N=256 tiles (far chunk only). Partition reductions (RMS ssq, softmax
den) are rank-1 matmuls; per-token broadcasts are one-hot-row matmuls;
reciprocals are batched and use the fast approx DVE op.
"""

import math
import numpy as np
from contextlib import ExitStack

import concourse.bass as bass
import concourse.mybir as mybir
import concourse.tile as tile
from concourse import bacc
from concourse.bass_utils import run_bass_kernel_spmd

F32 = mybir.dt.float32
F32R = mybir.dt.float32r
BF16 = mybir.dt.bfloat16
ADD = mybir.AluOpType.add
MULT = mybir.AluOpType.mult
EXP = mybir.ActivationFunctionType.Exp
SQRT = mybir.ActivationFunctionType.Sqrt
SQUARE = mybir.ActivationFunctionType.Square

EPS = 1e-8
NEG = -50.0  # additive pre-exp kill; exp(-50 + |s|max~11.4) ~ 2e-17


def full_cfg():
    return dict(B=2, S=2048, E=2048, D=128, G=2)


def derived(cfg):
    B, S, E, D, G = cfg["B"], cfg["S"], cfg["E"], cfg["D"], cfg["G"]
    NH = E // D            # 16 query heads
    ET = E // 128          # 16 contraction tiles
    NKT = S // 128         # 16 key tiles
    QPC = S // 4           # 512 query tokens per core
    CH = S // 8            # 256 chunk size
    TD = CH // 128         # 2 diagonal key-tiles per chunk
    GS = NH // G           # 8 heads per kv group
    return NH, ET, NKT, QPC, CH, TD, GS


def build_program(cfg, debug=False):
    B, S, E, D, G = cfg["B"], cfg["S"], cfg["E"], cfg["D"], cfg["G"]
    NH, ET, NKT, QPC, CH, TD, GS = derived(cfg)
    SCALE = 1.0 / math.sqrt(D)
    KC = 512
    NKC = S // KC          # 4 key-column chunks for K projection

    nc = bacc.Bacc()
    xT_d = nc.dram_tensor("xT", [E, S], BF16, kind="ExternalInput")
    wq_d = nc.dram_tensor("Wq", [E, E], BF16, kind="ExternalInput")
    wk_d = nc.dram_tensor("Wk", [E, G * D], BF16, kind="ExternalInput")
    wv_d = nc.dram_tensor("Wv", [E, G * D], BF16, kind="ExternalInput")
    wo_d = nc.dram_tensor("Wo", [E, E], BF16, kind="ExternalInput")
    bq_d = nc.dram_tensor("bq_t", [128, NH], F32, kind="ExternalInput")
    bk_d = nc.dram_tensor("bk_t", [128, G], F32, kind="ExternalInput")
    bv_d = nc.dram_tensor("bv_r", [1, G * D], BF16, kind="ExternalInput")
    bo_d = nc.dram_tensor("bo_t", [128, ET], F32, kind="ExternalInput")
    gq_d = nc.dram_tensor("gq_c", [128, 1], F32, kind="ExternalInput")
    gk_d = nc.dram_tensor("gk_c", [128, 1], F32, kind="ExternalInput")
    mask_d = nc.dram_tensor("mask2", [TD * 128, CH], BF16, kind="ExternalInput")
    oneh2_d = nc.dram_tensor("oneh2", [G, 256], BF16, kind="ExternalInput")
    oneh16_d = nc.dram_tensor("oneh16", [NH, NH * 128], BF16, kind="ExternalInput")
    ohT2f_d = nc.dram_tensor("ohT2f", [128, 2 * G], F32, kind="ExternalInput")
    ohT2b_d = nc.dram_tensor("ohT2b", [128, 2 * G], BF16, kind="ExternalInput")
    ohT16_d = nc.dram_tensor("ohT16", [128, NH * NH], F32, kind="ExternalInput")
    ones_d = nc.dram_tensor("ones1", [128, 1], F32, kind="ExternalInput")
    bcolB_d = nc.dram_tensor("bcolB", [128, 6], F32, kind="ExternalInput")
    akill_d = nc.dram_tensor("akill", [128, 6], F32, kind="ExternalInput")
    out_d = nc.dram_tensor("outT", [E, QPC], BF16, kind="ExternalOutput")
    if debug:
        dbg_pt_d = nc.dram_tensor("dbg_pt", [128, 3 * QPC], F32, kind="ExternalOutput")
        dbg_den_d = nc.dram_tensor("dbg_den", [1, QPC], F32, kind="ExternalOutput")
        dbg_cx_d = nc.dram_tensor("dbg_cx", [128, QPC], F32, kind="ExternalOutput")
        dbg_q_d = nc.dram_tensor("dbg_q", [128, QPC], F32, kind="ExternalOutput")
        dbg_k_d = nc.dram_tensor("dbg_k", [128, 2 * 128], F32, kind="ExternalOutput")
        dbg_ctx_d = nc.dram_tensor("dbg_ctx", [128, NH * QPC], F32, kind="ExternalOutput")
        dbg_b2_d = nc.dram_tensor("dbg_b2", [128, QPC], F32, kind="ExternalOutput")

    xT_r = xT_d.rearrange("(t p) s -> p t s", p=128)   # [128, ET, S]
    wq_r = wq_d.rearrange("(t p) c -> p t c", p=128)   # [128, ET, E]
    wk_r = wk_d.rearrange("(t p) c -> p t c", p=128)   # [128, ET, G*D]
    wv_r = wv_d.rearrange("(t p) c -> p t c", p=128)
    wo_r = wo_d.rearrange("(t p) c -> p t c", p=128)

    def r(ap):
        return ap if ap.dtype == F32R else ap.bitcast(F32R)

    with tile.TileContext(nc) as tc, ExitStack() as top:
        consts = top.enter_context(tc.tile_pool(name="consts", bufs=1))
        persist = top.enter_context(tc.tile_pool(name="persist", bufs=1))
        wkvp = top.enter_context(tc.tile_pool(name="wkv", bufs=1))

        # startup-critical DMAs first: x chunk 0 (fine-grained) and Wk/Wv
        xall = persist.tile([128, ET, S], BF16, tag="xall")
        for e4 in range(4):
            nc.sync.dma_start(
                out=xall[:, 4 * e4:4 * e4 + 4, 0:KC],
                in_=xT_r[:, 4 * e4:4 * e4 + 4, 0:KC])
        wk_sb = wkvp.tile([128, ET, G * D], BF16, tag="wk")
        for e4 in range(4):
            nc.sync.dma_start(out=wk_sb[:, 4 * e4:4 * e4 + 4, :],
                              in_=wk_r[:, 4 * e4:4 * e4 + 4, :])
        wv_sb = wkvp.tile([128, ET, G * D], BF16, tag="wv")
        nc.sync.dma_start(out=wv_sb, in_=wv_r)
        nc.sync.dma_start(out=xall[:, :, KC:2 * KC], in_=xT_r[:, :, KC:2 * KC])
        ones_col = consts.tile([128, 1], F32R)
        nc.sync.dma_start(out=ones_col, in_=ones_d[:, :].bitcast(F32R))
        ones_bf = consts.tile([128, 1], BF16)
        nc.vector.memset(ones_bf, 1.0)
        ones_rb = consts.tile([1, 128], BF16)
        nc.vector.memset(ones_rb, 1.0)
        ones_row = consts.tile([1, 128], BF16)
        nc.vector.memset(ones_row, 1.0)
        eps2 = consts.tile([G, 1], F32)
        nc.vector.memset(eps2, EPS)
        eps16 = consts.tile([NH, 1], F32)
        nc.vector.memset(eps16, EPS)
        gq_sb = consts.tile([128, 1], F32)
        nc.sync.dma_start(out=gq_sb, in_=gq_d[:, :])
        gk_sb = consts.tile([128, 1], F32)
        nc.sync.dma_start(out=gk_sb, in_=gk_d[:, :])
        bq_sb = consts.tile([128, NH], F32)
        nc.sync.dma_start(out=bq_sb, in_=bq_d[:, :])
        bk_sb = consts.tile([128, G], F32)
        nc.sync.dma_start(out=bk_sb, in_=bk_d[:, :])
        bv_sb = consts.tile([1, G * D], BF16)
        nc.sync.dma_start(out=bv_sb, in_=bv_d[:, :])
        bo_sb = consts.tile([128, ET], F32)
        nc.sync.dma_start(out=bo_sb, in_=bo_d[:, :])
        oneh2_sb = consts.tile([G, 256], BF16)
        nc.sync.dma_start(out=oneh2_sb, in_=oneh2_d[:, :])
        oneh16_sb = consts.tile([NH, NH * 128], BF16)
        nc.sync.dma_start(out=oneh16_sb, in_=oneh16_d[:, :])
        ohT2f_sb = consts.tile([128, 2 * G], F32R)
        nc.sync.dma_start(out=ohT2f_sb, in_=ohT2f_d[:, :].bitcast(F32R))
        ohT2b_sb = consts.tile([128, 2 * G], BF16)
        nc.sync.dma_start(out=ohT2b_sb, in_=ohT2b_d[:, :])
        ohT16_sb = consts.tile([128, NH * NH], F32R)
        nc.sync.dma_start(out=ohT16_sb, in_=ohT16_d[:, :].bitcast(F32R))
        bcolB_sb = consts.tile([128, 6], F32)
        nc.sync.dma_start(out=bcolB_sb, in_=bcolB_d[:, :])
        akill_sb = consts.tile([128, 6], F32)
        nc.sync.dma_start(out=akill_sb, in_=akill_d[:, :])
        mask_sb = []
        for t in range(TD):
            m = consts.tile([128, CH], BF16, tag=f"mask{t}", name=f"mask{t}")
            nc.sync.dma_start(out=m, in_=mask_d[t * 128:(t + 1) * 128, :])
            mask_sb.append(m)

        # bulk x (key chunks 2-3) after everything startup-critical
        for kc in range(2, NKC):
            nc.sync.dma_start(out=xall[:, :, kc * KC:(kc + 1) * KC],
                              in_=xT_r[:, :, kc * KC:(kc + 1) * KC])

        ktn = persist.tile([128, G, S], BF16, tag="ktn")
        vtok = persist.tile([128, NKT, G * D], BF16, tag="vtok")
        qtn = persist.tile([128, NH, QPC], BF16, tag="qtn")
        ctxt = persist.tile([128, NH, QPC], BF16, tag="ctxt")

        # ---------------- phase 1: K,V projections -----------------------
        with ExitStack() as p1:
            sqp = p1.enter_context(tc.tile_pool(name="sq1", bufs=3))
            smp = p1.enter_context(tc.tile_pool(name="sm1", bufs=2))
            pk = p1.enter_context(tc.tile_pool(name="pk", bufs=2, space="PSUM"))
            pv = p1.enter_context(tc.tile_pool(name="pv", bufs=2, space="PSUM"))
            pss = p1.enter_context(tc.tile_pool(name="pss", bufs=1, space="PSUM"))
            pbc = p1.enter_context(tc.tile_pool(name="pbc", bufs=2, space="PSUM"))

            # bv broadcast [128, G*D] in SBUF (outer product, done once)
            pbv = pbc.tile([128, G * D], F32, tag="pbc", name="pbv")
            nc.tensor.matmul(pbv, lhsT=ones_row, rhs=bv_sb,
                             start=True, stop=True)
            bvb_sb = wkvp.tile([128, G * D], F32, tag="bvb")
            nc.scalar.copy(out=bvb_sb, in_=pbv)

            pending = []

            def flush():
                while pending:
                    pending.pop(0)()

            for kc in range(NKC):
                ksl = slice(kc * KC, (kc + 1) * KC)
                # K projection, feature-major: out [128 (g*d), 512 tok]
                kaccs = []
                for g in range(G):
                    acc = pk.tile([128, KC], F32, tag="pk", name=f"kacc{g}")
                    for et in range(ET):
                        nc.tensor.matmul(
                            acc, lhsT=wk_sb[:, et, g * D:(g + 1) * D],
                            rhs=xall[:, et, ksl],
                            start=(et == 0), stop=(et == ET - 1))
                    kaccs.append(acc)
                # V projection, token-major: out [128 tok, G*D]
                vaccs = []
                for tb in range(4):
                    vacc = pv.tile([128, G * D], F32, tag="pv", name=f"vacc{tb}")
                    tsl = slice(kc * KC + tb * 128, kc * KC + (tb + 1) * 128)
                    for et in range(ET):
                        nc.tensor.matmul(
                            vacc, lhsT=xall[:, et, tsl], rhs=wv_sb[:, et, :],
                            start=(et == 0), stop=(et == ET - 1))
                    vaccs.append(vacc)

                def post_kv(kc=kc, ksl=ksl, kaccs=kaccs, vaccs=vaccs):
                    ss = pss.tile([G, KC], F32, tag="pss", name="ss")
                    for g in range(G):
                        # bias add, write raw K into ktn (bf16)
                        nc.vector.tensor_scalar(
                            out=ktn[:, g, ksl], in0=kaccs[g],
                            scalar1=bk_sb[:, g:g + 1], scalar2=None, op0=ADD)
                        sq = sqp.tile([128, KC], F32R, tag="sq", name="sq")
                        nc.scalar.activation(out=sq, in_=ktn[:, g, ksl],
                                             func=SQUARE)
                        nc.tensor.matmul(ss, lhsT=r(ohT2f_sb[:, 2 * g:2 * g + 2]),
                                         rhs=sq, start=(g == 0),
                                         stop=(g == G - 1))
                    srt = smp.tile([G, KC], F32, tag="srt", name="srt")
                    nc.scalar.activation(out=srt, in_=ss, func=SQRT,
                                         scale=1.0 / D, bias=eps2[:, :])
                    rk = smp.tile([G, KC], F32, tag="rk", name="rk")
                    nc.vector.reciprocal_approx_fast(out=rk, in_=srt)
                    rkb = smp.tile([G, KC], BF16, tag="rkb", name="rkb")
                    nc.vector.tensor_copy(out=rkb, in_=rk)
                    for g in range(G):
                        bc = pbc.tile([128, KC], F32, tag="pbc", name="bc")
                        nc.tensor.matmul(
                            bc, lhsT=oneh2_sb[:, g * 128:(g + 1) * 128],
                            rhs=rkb, start=True, stop=True)
                        # ktn = (ktn_raw * gk) * (1/rms)   in place
                        nc.vector.scalar_tensor_tensor(
                            out=ktn[:, g, ksl], in0=ktn[:, g, ksl],
                            scalar=gk_sb[:, :], in1=bc, op0=MULT, op1=MULT)
                    for tb in range(4):
                        kt = kc * 4 + tb
                        nc.vector.tensor_tensor(
                            out=vtok[:, kt, :], in0=vaccs[tb], in1=bvb_sb,
                            op=ADD)
                pending.append(post_kv)
                if kc >= 1:
                    pending.pop(0)()
            flush()

        # ---------------- phase 2: Q projection ---------------------------
        with ExitStack() as p2:
            wqp = p2.enter_context(tc.tile_pool(name="wqs", bufs=3))
            sqp2 = p2.enter_context(tc.tile_pool(name="sq2", bufs=3))
            smp2 = p2.enter_context(tc.tile_pool(name="sm2", bufs=1))
            pq = p2.enter_context(tc.tile_pool(name="pq", bufs=2, space="PSUM"))
            psq = p2.enter_context(tc.tile_pool(name="psq", bufs=1, space="PSUM"))
            pbc2p = p2.enter_context(tc.tile_pool(name="pbcq", bufs=2, space="PSUM"))

            ssq_all = psq.tile([NH, QPC], F32, tag="ssqa")
            pending2 = []
            wq_tiles = []
            for qc in range(min(2, NH)):
                w = wqp.tile([128, ET, 128], BF16, tag="wq", name="wq")
                nc.sync.dma_start(out=w,
                                  in_=wq_r[:, :, qc * 128:(qc + 1) * 128])
                wq_tiles.append(w)
            for qc in range(NH):
                if qc + 2 < NH:
                    w = wqp.tile([128, ET, 128], BF16, tag="wq", name="wq")
                    nc.sync.dma_start(
                        out=w, in_=wq_r[:, :, (qc + 2) * 128:(qc + 3) * 128])
                    wq_tiles.append(w)
                wq_sb = wq_tiles[qc]
                acc = pq.tile([128, QPC], F32, tag="pq", name="qacc")
                for et in range(ET):
                    nc.tensor.matmul(acc, lhsT=wq_sb[:, et, :],
                                     rhs=xall[:, et, 0:QPC],
                                     start=(et == 0), stop=(et == ET - 1))

                def post_q(qc=qc, acc=acc):
                    nc.vector.tensor_scalar(
                        out=qtn[:, qc, :], in0=acc,
                        scalar1=bq_sb[:, qc:qc + 1], scalar2=None, op0=ADD)
                    sq = sqp2.tile([128, QPC], F32R, tag="sq", name="qsq")
                    nc.scalar.activation(out=sq, in_=qtn[:, qc, :], func=SQUARE)
                    nc.tensor.matmul(
                        ssq_all, lhsT=r(ohT16_sb[:, qc * NH:(qc + 1) * NH]),
                        rhs=sq, start=(qc == 0), stop=(qc == NH - 1))
                pending2.append(post_q)
                if qc >= 1:
                    pending2.pop(0)()
            while pending2:
                pending2.pop(0)()

            srtq = smp2.tile([NH, QPC], F32, tag="srtq")
            nc.scalar.activation(out=srtq, in_=ssq_all, func=SQRT,
                                 scale=1.0 / D, bias=eps16[:, :])
            rq = smp2.tile([NH, QPC], F32, tag="rq")
            nc.vector.reciprocal_approx_fast(out=rq, in_=srtq)
            rqb = smp2.tile([NH, QPC], BF16, tag="rqb")
            nc.vector.tensor_copy(out=rqb, in_=rq)
            for qc in range(NH):
                bc = pbc2p.tile([128, QPC], F32, tag="pbcq", name="qbc")
                nc.tensor.matmul(
                    bc, lhsT=oneh16_sb[:, qc * 128:(qc + 1) * 128],
                    rhs=rqb, start=True, stop=True)
                nc.vector.scalar_tensor_tensor(
                    out=qtn[:, qc, :], in0=qtn[:, qc, :], scalar=gq_sb[:, :],
                    in1=bc, op0=MULT, op1=MULT)

        if debug:
            with tc.tile_pool(name="dbgp", bufs=1) as dbgp:
                tq = dbgp.tile([128, QPC], F32, tag="tq")
                nc.scalar.copy(out=tq, in_=qtn[:, 0, :])
                nc.sync.dma_start(out=dbg_q_d[:, :], in_=tq)
                tk = dbgp.tile([128, 256], F32, tag="tk")
                nc.scalar.copy(out=tk, in_=ktn[:, 0, 0:256])
                nc.sync.dma_start(out=dbg_k_d[:, :], in_=tk)

        # ---------------- phase 3: attention ------------------------------
        # per head: tiles 0,1 (N=512, A triangle-masked in place, B full),
        #           tiles 2,3 (N=256 B-only, triangle),
        #           tiles 4..9 (N=512, A kill-scalars), 10..15 (N=256, bias).
        # pt is always full [128,512] (A-half zeroed on narrow tiles) so den
        # and cx are single clean accumulation chains at N=512.
        with ExitStack() as p3:
            ptp = p3.enter_context(tc.tile_pool(name="pt", bufs=7))
            rdp = p3.enter_context(tc.tile_pool(name="rdp", bufs=2))
            psc = p3.enter_context(tc.tile_pool(name="psc", bufs=4, space="PSUM"))
            pden = p3.enter_context(tc.tile_pool(name="pden", bufs=2, space="PSUM"))
            pcx = p3.enter_context(tc.tile_pool(name="pcx", bufs=2, space="PSUM"))

            pend3 = []

            def flush3(lag=0):
                while len(pend3) > lag:
                    pend3.pop(0)()

            N512 = (0, 1, 4, 5, 6, 7, 8, 9)
            for h in range(NH):
                g = h // GS
                den = pden.tile([1, QPC], F32, tag="den", name="den")
                cx = pcx.tile([128, QPC], F32, tag="cx", name="cx")
                for t in range(NKT):
                    wide = t in N512
                    ksl = slice(t * 128, (t + 1) * 128)
                    if wide:
                        sc = psc.tile([128, QPC], F32, tag="sc", name="sc")
                        nc.tensor.matmul(sc, lhsT=ktn[:, g, ksl],
                                         rhs=qtn[:, h, :], start=True,
                                         stop=True)
                    else:
                        sc = psc.tile([128, QPC], F32, tag="sc", name="scn")
                        nc.tensor.matmul(sc[:, 0:CH], lhsT=ktn[:, g, ksl],
                                         rhs=qtn[:, h, CH:QPC], start=True,
                                         stop=True)

                    def post_t(h=h, g=g, t=t, wide=wide, sc=sc, den=den,
                               cx=cx):
                        if wide:
                            pt = ptp.tile([128, QPC], BF16, tag="pt",
                                          name="pt")
                            nc.scalar.activation(out=pt, in_=sc, func=EXP,
                                                 scale=SCALE)
                            if t < 2:
                                nc.vector.tensor_tensor(
                                    out=pt[:, 0:CH], in0=pt[:, 0:CH],
                                    in1=mask_sb[t], op=MULT)
                            else:
                                nc.vector.tensor_scalar(
                                    out=pt[:, 0:CH], in0=pt[:, 0:CH],
                                    scalar1=akill_sb[:, t - 4:t - 3],
                                    scalar2=None, op0=MULT)
                        else:
                            pt = ptp.tile([128, QPC], BF16, tag="pt",
                                          name="ptn")
                            nc.gpsimd.memset(pt[:, 0:CH], 0.0)
                            if t >= 10:
                                nc.scalar.activation(
                                    out=pt[:, CH:QPC], in_=sc[:, 0:CH],
                                    func=EXP, scale=SCALE,
                                    bias=bcolB_sb[:, t - 10:t - 9])
                            else:
                                nc.scalar.activation(out=pt[:, CH:QPC],
                                                     in_=sc[:, 0:CH],
                                                     func=EXP, scale=SCALE)
                                nc.vector.tensor_tensor(
                                    out=pt[:, CH:QPC], in0=pt[:, CH:QPC],
                                    in1=mask_sb[t - 2], op=MULT)
                        nc.tensor.matmul(den, lhsT=ones_bf, rhs=pt,
                                         start=(t == 0), stop=(t == NKT - 1))
                        nc.tensor.matmul(cx,
                                         lhsT=vtok[:, t, g * D:(g + 1) * D],
                                         rhs=pt, start=(t == 0),
                                         stop=(t == NKT - 1))
                    pend3.append(post_t)
                    flush3(lag=4)

                def post_head(h=h, den=den, cx=cx):
                    rd = rdp.tile([1, QPC], F32, tag="rd", name="rd")
                    nc.vector.reciprocal_approx_fast(out=rd, in_=den)
                    rdb = rdp.tile([1, QPC], BF16, tag="rdb", name="rdb")
                    nc.vector.tensor_copy(out=rdb, in_=rd)
                    b2 = psc.tile([128, QPC], F32, tag="sc", name="b2")
                    nc.tensor.matmul(b2, lhsT=ones_rb, rhs=rdb,
                                     start=True, stop=True)
                    if debug and h == 0:
                        td = rdp.tile([1, QPC], F32, tag="dbgden")
                        nc.vector.tensor_copy(out=td, in_=den)
                        nc.sync.dma_start(out=dbg_den_d[:, :], in_=td)
                        tcx = rdp.tile([128, QPC], F32, tag="dbgcx")
                        nc.scalar.copy(out=tcx, in_=cx)
                        nc.sync.dma_start(out=dbg_cx_d[:, :], in_=tcx)
                    cxs = rdp.tile([128, QPC], BF16, tag="cxs", name="cxs")
                    nc.scalar.copy(out=cxs, in_=cx)
                    nc.vector.tensor_tensor(out=ctxt[:, h, :], in0=cxs,
                                            in1=b2, op=MULT)
                pend3.append(post_head)
            flush3()

        # ---------------- phase 4: output projection ----------------------
        with ExitStack() as p4:
            wop = p4.enter_context(tc.tile_pool(name="wos", bufs=3))
            osb = p4.enter_context(tc.tile_pool(name="osb", bufs=3))
            po = p4.enter_context(tc.tile_pool(name="po", bufs=3, space="PSUM"))
            pend4 = []
            wo_tiles = []
            for c2 in range(min(2, ET)):
                w = wop.tile([128, ET, 128], BF16, tag="wo", name="wo")
                nc.sync.dma_start(out=w,
                                  in_=wo_r[:, :, c2 * 128:(c2 + 1) * 128])
                wo_tiles.append(w)
            for c2 in range(ET):
                if c2 + 2 < ET:
                    w = wop.tile([128, ET, 128], BF16, tag="wo", name="wo")
                    nc.sync.dma_start(
                        out=w, in_=wo_r[:, :, (c2 + 2) * 128:(c2 + 3) * 128])
                    wo_tiles.append(w)
                wo_sb = wo_tiles[c2]
                acc = po.tile([128, QPC], F32, tag="po", name="oacc")
                for ct in range(ET):
                    nc.tensor.matmul(acc, lhsT=wo_sb[:, ct, :],
                                     rhs=ctxt[:, ct, :],
                                     start=(ct == 0), stop=(ct == ET - 1))

                def post_o(c2=c2, acc=acc):
                    ot = osb.tile([128, QPC], BF16, tag="ot", name="ot")
                    nc.vector.tensor_scalar(
                        out=ot, in0=acc, scalar1=bo_sb[:, c2:c2 + 1],
                        scalar2=None, op0=ADD)
                    nc.sync.dma_start(
                        out=out_d[c2 * 128:(c2 + 1) * 128, :], in_=ot)
                pend4.append(post_o)
                if c2 >= 1:
                    pend4.pop(0)()
            while pend4:
                pend4.pop(0)()
    nc.compile()
    return nc


# ---------------------------------------------------------------------------
# host-side sharding
# ---------------------------------------------------------------------------

def core_perm(cfg, j):
    """Permutation of token positions for quarter j: [A | B | c1 | c2 | c3]."""
    S = cfg["S"]
    CH = S // 8
    A = np.arange(CH * j, CH * (j + 1))
    Bc = np.arange(S - CH * (j + 1), S - CH * j)
    rest = np.setdiff1d(np.arange(S), np.concatenate([A, Bc]))
    c1 = rest[rest < CH * j]
    c3 = rest[rest >= S - CH * j]
    c2 = rest[(rest >= CH * j) & (rest < S - CH * j)]
    perm = np.concatenate([A, Bc, c1, c2, c3])
    assert perm.shape == (S,)
    return perm


def tri_masks(cfg, dtype):
    S = cfg["S"]
    CH = S // 8
    TD = CH // 128
    m = np.zeros((TD * 128, CH), np.float32)
    for t in range(TD):
        kk = np.arange(128)[:, None] + t * 128
        qq = np.arange(CH)[None, :]
        m[t * 128:(t + 1) * 128, :] = (kk <= qq).astype(np.float32)
    return m.astype(dtype)


def make_in_maps(cfg, inputs):
    import ml_dtypes
    bf16 = ml_dtypes.bfloat16
    B, S, E, D, G = cfg["B"], cfg["S"], cfg["E"], cfg["D"], cfg["G"]
    NH, ET, NKT, QPC, CH, TD, GS = derived(cfg)
    x = np.asarray(inputs["x"], np.float32)

    oneh2 = np.zeros((G, 256), np.float32)
    oneh2[0, 0:128] = 1.0
    oneh2[1, 128:256] = 1.0
    oneh16 = np.zeros((NH, NH * 128), np.float32)
    for h in range(NH):
        oneh16[h, h * 128:(h + 1) * 128] = 1.0
    # stationary one-hot columns: slice [:, r*M:(r+1)*M] has ones in col r,
    # landing a rank-1 reduction on row r of an M-row PSUM tile
    ohT2 = np.zeros((128, 2 * G), np.float32)
    ohT2[:, 0] = 1.0
    ohT2[:, 3] = 1.0
    ohT16 = np.zeros((128, NH * NH), np.float32)
    for qc in range(NH):
        ohT16[:, qc * NH + qc] = 1.0

    shared = dict(
        Wq=np.ascontiguousarray(inputs["Wq"]).astype(bf16),
        Wk=np.ascontiguousarray(inputs["Wk"]).astype(bf16),
        Wv=np.ascontiguousarray(inputs["Wv"]).astype(bf16),
        Wo=np.ascontiguousarray(inputs["Wo"]).astype(bf16),
        bq_t=np.ascontiguousarray(
            np.asarray(inputs["bq"], np.float32).reshape(NH, 128).T),
        bk_t=np.ascontiguousarray(
            np.asarray(inputs["bk"], np.float32).reshape(G, 128).T),
        bv_r=np.asarray(inputs["bv"], np.float32).reshape(1, G * D).astype(bf16),
        bo_t=np.ascontiguousarray(
            np.asarray(inputs["bo"], np.float32).reshape(ET, 128).T),
        gq_c=np.asarray(inputs["gamma_q"], np.float32).reshape(128, 1).copy(),
        gk_c=np.asarray(inputs["gamma_k"], np.float32).reshape(128, 1).copy(),
        mask2=tri_masks(cfg, bf16),
        oneh2=oneh2.astype(bf16),
        oneh16=oneh16.astype(bf16),
        ohT2f=ohT2,
        ohT2b=ohT2.astype(bf16),
        ohT16=ohT16,
        ones1=np.ones((128, 1), np.float32),
    )
    in_maps, perms = [], []
    for c in range(8):
        b, j = c // 4, c % 4
        perm = core_perm(cfg, j)
        xt = np.ascontiguousarray(x[b].T[:, perm]).astype(bf16)
        # tiles 4..9 hold c1 (tokens before A): tile 4+i valid for A iff i < 2j
        ak = np.zeros((128, 6), np.float32)
        ak[:, 0:2 * j] = 1.0
        # tiles 10..15: valid for B iff t < 16-2j  (c2 vs c3)
        bc = np.full((128, 6), NEG, np.float32)
        bc[:, 0:6 - 2 * j] = 0.0
        m = dict(shared)
        m["xT"] = xt
        m["akill"] = ak
        m["bcolB"] = bc
        in_maps.append(m)
        perms.append(perm)
    return in_maps, perms


def assemble(cfg, results, perms):
    B, S, E = cfg["B"], cfg["S"], cfg["E"]
    QPC = S // 4
    out = np.empty((B, S, E), np.float32)
    for c in range(8):
        b = c // 4
        out[b, perms[c][:QPC], :] = np.asarray(results[c]["outT"], np.float32).T
    return out


_CACHE = {}


def kernel(**inputs):
    cfg = full_cfg()
    if "nc" not in _CACHE:
        _CACHE["nc"] = build_program(cfg)
    nc = _CACHE["nc"]
    in_maps, perms = make_in_maps(cfg, inputs)
    res = run_bass_kernel_spmd(nc, in_maps, list(range(8)))
    return assemble(cfg, res.results, perms)


# revision 26
# speedup vs baseline: 1.1821x; 1.1821x over previous
"""GQA attention block (RMSNorm-QK, causal, GQA) on 8 trn2 NeuronCores.

Sequence sharding, zero collectives. Core c handles batch c//4 and two
causally-balanced query chunks (j and 7-j of 8, 256 rows each). Host permutes
the key axis per core: [A | B | c1 | c2 | c3] so queries sit at columns
[0:512] and causality is data (two fixed triangle masks + per-core 0/1
column-kill scalars + exp bias columns); one uniform SPMD program serves all
cores.

All matmul operands are bf16 (fp32 PSUM accumulation). Activations are
feature-major ([feature, token]); V is projected token-major directly
(lhsT = x tile), so no on-device transposes at all. Scores are [k, q];
softmax needs no max subtraction (RMS-normalized q,k bound |s|/sqrt(D)).
Attention is causally trimmed: per head 8 N=512 score tiles (both chunks)
+ 8 N=256 tiles (far chunk only). Partition reductions (RMS ssq, softmax
den) are rank-1 matmuls; per-token broadcasts are one-hot-row matmuls;
reciprocals are batched and use the fast approx DVE op.
"""

import math
import numpy as np
from contextlib import ExitStack

import concourse.bass as bass
import concourse.mybir as mybir
import concourse.tile as tile
from concourse import bacc
from concourse.bass_utils import run_bass_kernel_spmd

F32 = mybir.dt.float32
F32R = mybir.dt.float32r
BF16 = mybir.dt.bfloat16
ADD = mybir.AluOpType.add
MULT = mybir.AluOpType.mult
EXP = mybir.ActivationFunctionType.Exp
SQRT = mybir.ActivationFunctionType.Sqrt
SQUARE = mybir.ActivationFunctionType.Square

EPS = 1e-8
NEG = -50.0  # additive pre-exp kill; exp(-50 + |s|max~11.4) ~ 2e-17


def full_cfg():
    return dict(B=2, S=2048, E=2048, D=128, G=2)


def derived(cfg):
    B, S, E, D, G = cfg["B"], cfg["S"], cfg["E"], cfg["D"], cfg["G"]
    NH = E // D            # 16 query heads
    ET = E // 128          # 16 contraction tiles
    NKT = S // 128         # 16 key tiles
    QPC = S // 4           # 512 query tokens per core
    CH = S // 8            # 256 chunk size
    TD = CH // 128         # 2 diagonal key-tiles per chunk
    GS = NH // G           # 8 heads per kv group
    return NH, ET, NKT, QPC, CH, TD, GS


def build_program(cfg, debug=False):
    B, S, E, D, G = cfg["B"], cfg["S"], cfg["E"], cfg["D"], cfg["G"]
    NH, ET, NKT, QPC, CH, TD, GS = derived(cfg)
    SCALE = 1.0 / math.sqrt(D)
    KC = 512
    NKC = S // KC          # 4 key-column chunks for K projection

    nc = bacc.Bacc()
    xT_d = nc.dram_tensor("xT", [E, S], BF16, kind="ExternalInput")
    wq_d = nc.dram_tensor("Wq", [E, E], BF16, kind="ExternalInput")
    wk_d = nc.dram_tensor("Wk", [E, G * D], BF16, kind="ExternalInput")
    wv_d = nc.dram_tensor("Wv", [E, G * D], BF16, kind="ExternalInput")
    wo_d = nc.dram_tensor("Wo", [E, E], BF16, kind="ExternalInput")
    bq_d = nc.dram_tensor("bq_t", [128, NH], F32, kind="ExternalInput")
    bk_d = nc.dram_tensor("bk_t", [128, G], F32, kind="ExternalInput")
    bv_d = nc.dram_tensor("bv_r", [1, G * D], BF16, kind="ExternalInput")
    bo_d = nc.dram_tensor("bo_t", [128, ET], F32, kind="ExternalInput")
    gq_d = nc.dram_tensor("gq_c", [128, 1], F32, kind="ExternalInput")
    gk_d = nc.dram_tensor("gk_c", [128, 1], F32, kind="ExternalInput")
    mask_d = nc.dram_tensor("mask2", [TD * 128, CH], BF16, kind="ExternalInput")
    oneh2_d = nc.dram_tensor("oneh2", [G, 256], BF16, kind="ExternalInput")
    oneh16_d = nc.dram_tensor("oneh16", [NH, NH * 128], BF16, kind="ExternalInput")
    ohT2f_d = nc.dram_tensor("ohT2f", [128, 2 * G], F32, kind="ExternalInput")
    ohT2b_d = nc.dram_tensor("ohT2b", [128, 2 * G], BF16, kind="ExternalInput")
    ohT16_d = nc.dram_tensor("ohT16", [128, NH * NH], F32, kind="ExternalInput")
    ones_d = nc.dram_tensor("ones1", [128, 1], F32, kind="ExternalInput")
    bcolB_d = nc.dram_tensor("bcolB", [128, 6], F32, kind="ExternalInput")
    akill_d = nc.dram_tensor("akill", [128, 6], F32, kind="ExternalInput")
    out_d = nc.dram_tensor("outT", [E, QPC], BF16, kind="ExternalOutput")
    if debug:
        dbg_pt_d = nc.dram_tensor("dbg_pt", [128, 3 * QPC], F32, kind="ExternalOutput")
        dbg_den_d = nc.dram_tensor("dbg_den", [1, QPC], F32, kind="ExternalOutput")
        dbg_cx_d = nc.dram_tensor("dbg_cx", [128, QPC], F32, kind="ExternalOutput")
        dbg_q_d = nc.dram_tensor("dbg_q", [128, QPC], F32, kind="ExternalOutput")
        dbg_k_d = nc.dram_tensor("dbg_k", [128, 2 * 128], F32, kind="ExternalOutput")
        dbg_ctx_d = nc.dram_tensor("dbg_ctx", [128, NH * QPC], F32, kind="ExternalOutput")
        dbg_b2_d = nc.dram_tensor("dbg_b2", [128, QPC], F32, kind="ExternalOutput")

    xT_r = xT_d.rearrange("(t p) s -> p t s", p=128)   # [128, ET, S]
    wq_r = wq_d.rearrange("(t p) c -> p t c", p=128)   # [128, ET, E]
    wk_r = wk_d.rearrange("(t p) c -> p t c", p=128)   # [128, ET, G*D]
    wv_r = wv_d.rearrange("(t p) c -> p t c", p=128)
    wo_r = wo_d.rearrange("(t p) c -> p t c", p=128)

    def r(ap):
        return ap if ap.dtype == F32R else ap.bitcast(F32R)

    with tile.TileContext(nc) as tc, ExitStack() as top:
        consts = top.enter_context(tc.tile_pool(name="consts", bufs=1))
        persist = top.enter_context(tc.tile_pool(name="persist", bufs=1))
        wkvp = top.enter_context(tc.tile_pool(name="wkv", bufs=1))

        # startup-critical DMAs first: x chunk 0 (fine-grained) and Wk/Wv
        xall = persist.tile([128, ET, S], BF16, tag="xall")
        for e4 in range(4):
            nc.sync.dma_start(
                out=xall[:, 4 * e4:4 * e4 + 4, 0:KC],
                in_=xT_r[:, 4 * e4:4 * e4 + 4, 0:KC])
        wk_sb = wkvp.tile([128, ET, G * D], BF16, tag="wk")
        for e4 in range(4):
            nc.sync.dma_start(out=wk_sb[:, 4 * e4:4 * e4 + 4, :],
                              in_=wk_r[:, 4 * e4:4 * e4 + 4, :])
        wv_sb = wkvp.tile([128, ET, G * D], BF16, tag="wv")
        nc.sync.dma_start(out=wv_sb, in_=wv_r)
        nc.sync.dma_start(out=xall[:, :, KC:2 * KC], in_=xT_r[:, :, KC:2 * KC])
        ones_col = consts.tile([128, 1], F32R)
        nc.sync.dma_start(out=ones_col, in_=ones_d[:, :].bitcast(F32R))
        ones_bf = consts.tile([128, 1], BF16)
        nc.vector.memset(ones_bf, 1.0)
        ones_rb = consts.tile([1, 128], BF16)
        nc.vector.memset(ones_rb, 1.0)
        ones_row = consts.tile([1, 128], BF16)
        nc.vector.memset(ones_row, 1.0)
        eps2 = consts.tile([G, 1], F32)
        nc.vector.memset(eps2, EPS)
        eps16 = consts.tile([NH, 1], F32)
        nc.vector.memset(eps16, EPS)
        gq_sb = consts.tile([128, 1], F32)
        nc.sync.dma_start(out=gq_sb, in_=gq_d[:, :])
        gk_sb = consts.tile([128, 1], F32)
        nc.sync.dma_start(out=gk_sb, in_=gk_d[:, :])
        bq_sb = consts.tile([128, NH], F32)
        nc.sync.dma_start(out=bq_sb, in_=bq_d[:, :])
        bk_sb = consts.tile([128, G], F32)
        nc.sync.dma_start(out=bk_sb, in_=bk_d[:, :])
        bv_sb = consts.tile([1, G * D], BF16)
        nc.sync.dma_start(out=bv_sb, in_=bv_d[:, :])
        bo_sb = consts.tile([128, ET], F32)
        nc.sync.dma_start(out=bo_sb, in_=bo_d[:, :])
        oneh2_sb = consts.tile([G, 256], BF16)
        nc.sync.dma_start(out=oneh2_sb, in_=oneh2_d[:, :])
        oneh16_sb = consts.tile([NH, NH * 128], BF16)
        nc.sync.dma_start(out=oneh16_sb, in_=oneh16_d[:, :])
        ohT2f_sb = consts.tile([128, 2 * G], F32R)
        nc.sync.dma_start(out=ohT2f_sb, in_=ohT2f_d[:, :].bitcast(F32R))
        ohT2b_sb = consts.tile([128, 2 * G], BF16)
        nc.sync.dma_start(out=ohT2b_sb, in_=ohT2b_d[:, :])
        ohT16_sb = consts.tile([128, NH * NH], F32R)
        nc.sync.dma_start(out=ohT16_sb, in_=ohT16_d[:, :].bitcast(F32R))
        bcolB_sb = consts.tile([128, 6], F32)
        nc.sync.dma_start(out=bcolB_sb, in_=bcolB_d[:, :])
        akill_sb = consts.tile([128, 6], F32)
        nc.sync.dma_start(out=akill_sb, in_=akill_d[:, :])
        mask_sb = []
        for t in range(TD):
            m = consts.tile([128, CH], BF16, tag=f"mask{t}", name=f"mask{t}")
            nc.sync.dma_start(out=m, in_=mask_d[t * 128:(t + 1) * 128, :])
            mask_sb.append(m)

        # bulk x (key chunks 2-3) after everything startup-critical
        for kc in range(2, NKC):
            nc.sync.dma_start(out=xall[:, :, kc * KC:(kc + 1) * KC],
                              in_=xT_r[:, :, kc * KC:(kc + 1) * KC])

        ktn = persist.tile([128, G, S], BF16, tag="ktn")
        vtok = persist.tile([128, NKT, G * D], BF16, tag="vtok")
        qtn = persist.tile([128, NH, QPC], BF16, tag="qtn")
        ctxt = persist.tile([128, NH, QPC], BF16, tag="ctxt")

        # ---------------- phase 1: K,V projections -----------------------
        with ExitStack() as p1:
            sqp = p1.enter_context(tc.tile_pool(name="sq1", bufs=3))
            smp = p1.enter_context(tc.tile_pool(name="sm1", bufs=2))
            pk = p1.enter_context(tc.tile_pool(name="pk", bufs=2, space="PSUM"))
            pv = p1.enter_context(tc.tile_pool(name="pv", bufs=2, space="PSUM"))
            pss = p1.enter_context(tc.tile_pool(name="pss", bufs=1, space="PSUM"))
            pbc = p1.enter_context(tc.tile_pool(name="pbc", bufs=2, space="PSUM"))

            # bv broadcast [128, G*D] in SBUF (outer product, done once)
            pbv = pbc.tile([128, G * D], F32, tag="pbc", name="pbv")
            nc.tensor.matmul(pbv, lhsT=ones_row, rhs=bv_sb,
                             start=True, stop=True)
            bvb_sb = wkvp.tile([128, G * D], F32, tag="bvb")
            nc.scalar.copy(out=bvb_sb, in_=pbv)

            pending = []

            def flush():
                while pending:
                    pending.pop(0)()

            for kc in range(NKC):
                ksl = slice(kc * KC, (kc + 1) * KC)
                # K projection, feature-major: out [128 (g*d), 512 tok]
                kaccs = []
                for g in range(G):
                    acc = pk.tile([128, KC], F32, tag="pk", name=f"kacc{g}")
                    for et in range(ET):
                        nc.tensor.matmul(
                            acc, lhsT=wk_sb[:, et, g * D:(g + 1) * D],
                            rhs=xall[:, et, ksl],
                            start=(et == 0), stop=(et == ET - 1))
                    kaccs.append(acc)
                # V projection, token-major: out [128 tok, G*D]
                vaccs = []
                for tb in range(4):
                    vacc = pv.tile([128, G * D], F32, tag="pv", name=f"vacc{tb}")
                    tsl = slice(kc * KC + tb * 128, kc * KC + (tb + 1) * 128)
                    for et in range(ET):
                        nc.tensor.matmul(
                            vacc, lhsT=xall[:, et, tsl], rhs=wv_sb[:, et, :],
                            start=(et == 0), stop=(et == ET - 1))
                    vaccs.append(vacc)

                def post_kv(kc=kc, ksl=ksl, kaccs=kaccs, vaccs=vaccs):
                    ss = pss.tile([G, KC], F32, tag="pss", name="ss")
                    for g in range(G):
                        # bias add, write raw K into ktn (bf16)
                        nc.vector.tensor_scalar(
                            out=ktn[:, g, ksl], in0=kaccs[g],
                            scalar1=bk_sb[:, g:g + 1], scalar2=None, op0=ADD)
                        sq = sqp.tile([128, KC], F32R, tag="sq", name="sq")
                        nc.scalar.activation(out=sq, in_=ktn[:, g, ksl],
                                             func=SQUARE)
                        nc.tensor.matmul(ss, lhsT=r(ohT2f_sb[:, 2 * g:2 * g + 2]),
                                         rhs=sq, start=(g == 0),
                                         stop=(g == G - 1))
                    srt = smp.tile([G, KC], F32, tag="srt", name="srt")
                    nc.scalar.activation(out=srt, in_=ss, func=SQRT,
                                         scale=1.0 / D, bias=eps2[:, :])
                    rk = smp.tile([G, KC], F32, tag="rk", name="rk")
                    nc.vector.reciprocal_approx_fast(out=rk, in_=srt)
                    rkb = smp.tile([G, KC], BF16, tag="rkb", name="rkb")
                    nc.vector.tensor_copy(out=rkb, in_=rk)
                    for g in range(G):
                        bc = pbc.tile([128, KC], F32, tag="pbc", name="bc")
                        nc.tensor.matmul(
                            bc, lhsT=oneh2_sb[:, g * 128:(g + 1) * 128],
                            rhs=rkb, start=True, stop=True)
                        # ktn = (ktn_raw * gk) * (1/rms)   in place
                        nc.vector.scalar_tensor_tensor(
                            out=ktn[:, g, ksl], in0=ktn[:, g, ksl],
                            scalar=gk_sb[:, :], in1=bc, op0=MULT, op1=MULT)
                    for tb in range(4):
                        kt = kc * 4 + tb
                        nc.vector.tensor_tensor(
                            out=vtok[:, kt, :], in0=vaccs[tb], in1=bvb_sb,
                            op=ADD)
                pending.append(post_kv)
                if kc >= 1:
                    pending.pop(0)()
            flush()

        # ---------------- phase 2: Q projection ---------------------------
        with ExitStack() as p2:
            wqp = p2.enter_context(tc.tile_pool(name="wqs", bufs=3))
            sqp2 = p2.enter_context(tc.tile_pool(name="sq2", bufs=3))
            smp2 = p2.enter_context(tc.tile_pool(name="sm2", bufs=1))
            pq = p2.enter_context(tc.tile_pool(name="pq", bufs=2, space="PSUM"))
            psq = p2.enter_context(tc.tile_pool(name="psq", bufs=1, space="PSUM"))
            pbc2p = p2.enter_context(tc.tile_pool(name="pbcq", bufs=2, space="PSUM"))

            ssq_all = psq.tile([NH, QPC], F32, tag="ssqa")
            pending2 = []
            wq_tiles = []
            for qc in range(min(2, NH)):
                w = wqp.tile([128, ET, 128], BF16, tag="wq", name="wq")
                nc.sync.dma_start(out=w,
                                  in_=wq_r[:, :, qc * 128:(qc + 1) * 128])
                wq_tiles.append(w)
            for qc in range(NH):
                if qc + 2 < NH:
                    w = wqp.tile([128, ET, 128], BF16, tag="wq", name="wq")
                    nc.sync.dma_start(
                        out=w, in_=wq_r[:, :, (qc + 2) * 128:(qc + 3) * 128])
                    wq_tiles.append(w)
                wq_sb = wq_tiles[qc]
                acc = pq.tile([128, QPC], F32, tag="pq", name="qacc")
                for et in range(ET):
                    nc.tensor.matmul(acc, lhsT=wq_sb[:, et, :],
                                     rhs=xall[:, et, 0:QPC],
                                     start=(et == 0), stop=(et == ET - 1))

                def post_q(qc=qc, acc=acc):
                    nc.vector.tensor_scalar(
                        out=qtn[:, qc, :], in0=acc,
                        scalar1=bq_sb[:, qc:qc + 1], scalar2=None, op0=ADD)
                    sq = sqp2.tile([128, QPC], F32R, tag="sq", name="qsq")
                    nc.scalar.activation(out=sq, in_=qtn[:, qc, :], func=SQUARE)
                    nc.tensor.matmul(
                        ssq_all, lhsT=r(ohT16_sb[:, qc * NH:(qc + 1) * NH]),
                        rhs=sq, start=(qc == 0), stop=(qc == NH - 1))
                pending2.append(post_q)
                if qc >= 1:
                    pending2.pop(0)()
            while pending2:
                pending2.pop(0)()

            srtq = smp2.tile([NH, QPC], F32, tag="srtq")
            nc.scalar.activation(out=srtq, in_=ssq_all, func=SQRT,
                                 scale=1.0 / D, bias=eps16[:, :])
            rq = smp2.tile([NH, QPC], F32, tag="rq")
            nc.vector.reciprocal_approx_fast(out=rq, in_=srtq)
            rqb = smp2.tile([NH, QPC], BF16, tag="rqb")
            nc.vector.tensor_copy(out=rqb, in_=rq)
            for qc in range(NH):
                bc = pbc2p.tile([128, QPC], F32, tag="pbcq", name="qbc")
                nc.tensor.matmul(
                    bc, lhsT=oneh16_sb[:, qc * 128:(qc + 1) * 128],
                    rhs=rqb, start=True, stop=True)
                nc.vector.scalar_tensor_tensor(
                    out=qtn[:, qc, :], in0=qtn[:, qc, :], scalar=gq_sb[:, :],
                    in1=bc, op0=MULT, op1=MULT)

        if debug:
            with tc.tile_pool(name="dbgp", bufs=1) as dbgp:
                tq = dbgp.tile([128, QPC], F32, tag="tq")
                nc.scalar.copy(out=tq, in_=qtn[:, 0, :])
                nc.sync.dma_start(out=dbg_q_d[:, :], in_=tq)
                tk = dbgp.tile([128, 256], F32, tag="tk")
                nc.scalar.copy(out=tk, in_=ktn[:, 0, 0:256])
                nc.sync.dma_start(out=dbg_k_d[:, :], in_=tk)

        # ---------------- phase 3: attention ------------------------------
        # per head: tiles 0,1 (N=512, A triangle-masked in place, B full),
        #           tiles 2,3 (N=256 B-only, triangle),
        #           tiles 4..9 (N=512, A kill-scalars), 10..15 (N=256, bias).
        # pt is always full [128,512] (A-half zeroed on narrow tiles) so den
        # and cx are single clean accumulation chains at N=512.
        with ExitStack() as p3:
            ptp = p3.enter_context(tc.tile_pool(name="pt", bufs=5))
            rdp = p3.enter_context(tc.tile_pool(name="rdp", bufs=2))
            psc = p3.enter_context(tc.tile_pool(name="psc", bufs=4, space="PSUM"))
            pden = p3.enter_context(tc.tile_pool(name="pden", bufs=2, space="PSUM"))
            pcx = p3.enter_context(tc.tile_pool(name="pcx", bufs=2, space="PSUM"))

            pend3 = []

            def flush3(lag=0):
                while len(pend3) > lag:
                    pend3.pop(0)()

            N512 = (0, 1, 4, 5, 6, 7, 8, 9)
            for h in range(NH):
                g = h // GS
                den = pden.tile([1, QPC], F32, tag="den", name="den")
                cx = pcx.tile([128, QPC], F32, tag="cx", name="cx")
                for t in range(NKT):
                    wide = t in N512
                    ksl = slice(t * 128, (t + 1) * 128)
                    if wide:
                        sc = psc.tile([128, QPC], F32, tag="sc", name="sc")
                        nc.tensor.matmul(sc, lhsT=ktn[:, g, ksl],
                                         rhs=qtn[:, h, :], start=True,
                                         stop=True)
                    else:
                        sc = psc.tile([128, QPC], F32, tag="sc", name="scn")
                        nc.tensor.matmul(sc[:, 0:CH], lhsT=ktn[:, g, ksl],
                                         rhs=qtn[:, h, CH:QPC], start=True,
                                         stop=True)

                    def post_t(h=h, g=g, t=t, wide=wide, sc=sc, den=den,
                               cx=cx):
                        if wide:
                            pt = ptp.tile([128, QPC], BF16, tag="pt",
                                          name="pt")
                            nc.scalar.activation(out=pt, in_=sc, func=EXP,
                                                 scale=SCALE)
                            if t < 2:
                                nc.vector.tensor_tensor(
                                    out=pt[:, 0:CH], in0=pt[:, 0:CH],
                                    in1=mask_sb[t], op=MULT)
                            else:
                                nc.vector.tensor_scalar(
                                    out=pt[:, 0:CH], in0=pt[:, 0:CH],
                                    scalar1=akill_sb[:, t - 4:t - 3],
                                    scalar2=None, op0=MULT)
                        else:
                            pt = ptp.tile([128, QPC], BF16, tag="pt",
                                          name="ptn")
                            nc.gpsimd.memset(pt[:, 0:CH], 0.0)
                            if t >= 10:
                                nc.scalar.activation(
                                    out=pt[:, CH:QPC], in_=sc[:, 0:CH],
                                    func=EXP, scale=SCALE,
                                    bias=bcolB_sb[:, t - 10:t - 9])
                            else:
                                nc.scalar.activation(out=pt[:, CH:QPC],
                                                     in_=sc[:, 0:CH],
                                                     func=EXP, scale=SCALE)
                                nc.vector.tensor_tensor(
                                    out=pt[:, CH:QPC], in0=pt[:, CH:QPC],
                                    in1=mask_sb[t - 2], op=MULT)
                        nc.tensor.matmul(den, lhsT=ones_bf, rhs=pt,
                                         start=(t == 0), stop=(t == NKT - 1))
                        nc.tensor.matmul(cx,
                                         lhsT=vtok[:, t, g * D:(g + 1) * D],
                                         rhs=pt, start=(t == 0),
                                         stop=(t == NKT - 1))
                    pend3.append(post_t)
                    flush3(lag=3)

                def post_head(h=h, den=den, cx=cx):
                    rd = rdp.tile([1, QPC], F32, tag="rd", name="rd")
                    nc.vector.reciprocal_approx_fast(out=rd, in_=den)
                    rdb = rdp.tile([1, QPC], BF16, tag="rdb", name="rdb")
                    nc.vector.tensor_copy(out=rdb, in_=rd)
                    b2 = psc.tile([128, QPC], F32, tag="sc", name="b2")
                    nc.tensor.matmul(b2, lhsT=ones_rb, rhs=rdb,
                                     start=True, stop=True)
                    if debug and h == 0:
                        td = rdp.tile([1, QPC], F32, tag="dbgden")
                        nc.vector.tensor_copy(out=td, in_=den)
                        nc.sync.dma_start(out=dbg_den_d[:, :], in_=td)
                        tcx = rdp.tile([128, QPC], F32, tag="dbgcx")
                        nc.scalar.copy(out=tcx, in_=cx)
                        nc.sync.dma_start(out=dbg_cx_d[:, :], in_=tcx)
                    cxs = rdp.tile([128, QPC], BF16, tag="cxs", name="cxs")
                    nc.scalar.copy(out=cxs, in_=cx)
                    nc.vector.tensor_tensor(out=ctxt[:, h, :], in0=cxs,
                                            in1=b2, op=MULT)
                pend3.append(post_head)
            flush3()

        # ---------------- phase 4: output projection ----------------------
        with ExitStack() as p4:
            wop = p4.enter_context(tc.tile_pool(name="wos", bufs=3))
            osb = p4.enter_context(tc.tile_pool(name="osb", bufs=3))
            po = p4.enter_context(tc.tile_pool(name="po", bufs=3, space="PSUM"))
            pend4 = []
            wo_tiles = []
            for c2 in range(min(2, ET)):
                w = wop.tile([128, ET, 128], BF16, tag="wo", name="wo")
                nc.sync.dma_start(out=w,
                                  in_=wo_r[:, :, c2 * 128:(c2 + 1) * 128])
                wo_tiles.append(w)
            for c2 in range(ET):
                if c2 + 2 < ET:
                    w = wop.tile([128, ET, 128], BF16, tag="wo", name="wo")
                    nc.sync.dma_start(
                        out=w, in_=wo_r[:, :, (c2 + 2) * 128:(c2 + 3) * 128])
                    wo_tiles.append(w)
                wo_sb = wo_tiles[c2]
                acc = po.tile([128, QPC], F32, tag="po", name="oacc")
                for ct in range(ET):
                    nc.tensor.matmul(acc, lhsT=wo_sb[:, ct, :],
                                     rhs=ctxt[:, ct, :],
                                     start=(ct == 0), stop=(ct == ET - 1))

                def post_o(c2=c2, acc=acc):
                    ot = osb.tile([128, QPC], BF16, tag="ot", name="ot")
                    nc.vector.tensor_scalar(
                        out=ot, in0=acc, scalar1=bo_sb[:, c2:c2 + 1],
                        scalar2=None, op0=ADD)
                    nc.sync.dma_start(
                        out=out_d[c2 * 128:(c2 + 1) * 128, :], in_=ot)
                pend4.append(post_o)
                if c2 >= 1:
                    pend4.pop(0)()
            while pend4:
                pend4.pop(0)()
    nc.compile()
    return nc


# ---------------------------------------------------------------------------
# host-side sharding
# ---------------------------------------------------------------------------

def core_perm(cfg, j):
    """Permutation of token positions for quarter j: [A | B | c1 | c2 | c3]."""
    S = cfg["S"]
    CH = S // 8
    A = np.arange(CH * j, CH * (j + 1))
    Bc = np.arange(S - CH * (j + 1), S - CH * j)
    rest = np.setdiff1d(np.arange(S), np.concatenate([A, Bc]))
    c1 = rest[rest < CH * j]
    c3 = rest[rest >= S - CH * j]
    c2 = rest[(rest >= CH * j) & (rest < S - CH * j)]
    perm = np.concatenate([A, Bc, c1, c2, c3])
    assert perm.shape == (S,)
    return perm


def tri_masks(cfg, dtype):
    S = cfg["S"]
    CH = S // 8
    TD = CH // 128
    m = np.zeros((TD * 128, CH), np.float32)
    for t in range(TD):
        kk = np.arange(128)[:, None] + t * 128
        qq = np.arange(CH)[None, :]
        m[t * 128:(t + 1) * 128, :] = (kk <= qq).astype(np.float32)
    return m.astype(dtype)


def make_in_maps(cfg, inputs):
    import ml_dtypes
    bf16 = ml_dtypes.bfloat16
    B, S, E, D, G = cfg["B"], cfg["S"], cfg["E"], cfg["D"], cfg["G"]
    NH, ET, NKT, QPC, CH, TD, GS = derived(cfg)
    x = np.asarray(inputs["x"], np.float32)

    oneh2 = np.zeros((G, 256), np.float32)
    oneh2[0, 0:128] = 1.0
    oneh2[1, 128:256] = 1.0
    oneh16 = np.zeros((NH, NH * 128), np.float32)
    for h in range(NH):
        oneh16[h, h * 128:(h + 1) * 128] = 1.0
    # stationary one-hot columns: slice [:, r*M:(r+1)*M] has ones in col r,
    # landing a rank-1 reduction on row r of an M-row PSUM tile
    ohT2 = np.zeros((128, 2 * G), np.float32)
    ohT2[:, 0] = 1.0
    ohT2[:, 3] = 1.0
    ohT16 = np.zeros((128, NH * NH), np.float32)
    for qc in range(NH):
        ohT16[:, qc * NH + qc] = 1.0

    shared = dict(
        Wq=np.ascontiguousarray(inputs["Wq"]).astype(bf16),
        Wk=np.ascontiguousarray(inputs["Wk"]).astype(bf16),
        Wv=np.ascontiguousarray(inputs["Wv"]).astype(bf16),
        Wo=np.ascontiguousarray(inputs["Wo"]).astype(bf16),
        bq_t=np.ascontiguousarray(
            np.asarray(inputs["bq"], np.float32).reshape(NH, 128).T),
        bk_t=np.ascontiguousarray(
            np.asarray(inputs["bk"], np.float32).reshape(G, 128).T),
        bv_r=np.asarray(inputs["bv"], np.float32).reshape(1, G * D).astype(bf16),
        bo_t=np.ascontiguousarray(
            np.asarray(inputs["bo"], np.float32).reshape(ET, 128).T),
        gq_c=np.asarray(inputs["gamma_q"], np.float32).reshape(128, 1).copy(),
        gk_c=np.asarray(inputs["gamma_k"], np.float32).reshape(128, 1).copy(),
        mask2=tri_masks(cfg, bf16),
        oneh2=oneh2.astype(bf16),
        oneh16=oneh16.astype(bf16),
        ohT2f=ohT2,
        ohT2b=ohT2.astype(bf16),
        ohT16=ohT16,
        ones1=np.ones((128, 1), np.float32),
    )
    in_maps, perms = [], []
    for c in range(8):
        b, j = c // 4, c % 4
        perm = core_perm(cfg, j)
        xt = np.ascontiguousarray(x[b].T[:, perm]).astype(bf16)
        # tiles 4..9 hold c1 (tokens before A): tile 4+i valid for A iff i < 2j
        ak = np.zeros((128, 6), np.float32)
        ak[:, 0:2 * j] = 1.0
        # tiles 10..15: valid for B iff t < 16-2j  (c2 vs c3)
        bc = np.full((128, 6), NEG, np.float32)
        bc[:, 0:6 - 2 * j] = 0.0
        m = dict(shared)
        m["xT"] = xt
        m["akill"] = ak
        m["bcolB"] = bc
        in_maps.append(m)
        perms.append(perm)
    return in_maps, perms


def assemble(cfg, results, perms):
    B, S, E = cfg["B"], cfg["S"], cfg["E"]
    QPC = S // 4
    out = np.empty((B, S, E), np.float32)
    for c in range(8):
        b = c // 4
        out[b, perms[c][:QPC], :] = np.asarray(results[c]["outT"], np.float32).T
    return out


_CACHE = {}


def kernel(**inputs):
    cfg = full_cfg()
    if "nc" not in _CACHE:
        _CACHE["nc"] = build_program(cfg)
    nc = _CACHE["nc"]
    in_maps, perms = make_in_maps(cfg, inputs)
    res = run_bass_kernel_spmd(nc, in_maps, list(range(8)))
    return assemble(cfg, res.results, perms)


# revision 28
# speedup vs baseline: 1.1873x; 1.0044x over previous
"""GQA attention block (RMSNorm-QK, causal, GQA) on 8 trn2 NeuronCores.

Sequence sharding, zero collectives. Core c handles batch c//4 and two
causally-balanced query chunks (j and 7-j of 8, 256 rows each). Host permutes
the key axis per core: [A | B | c1 | c2 | c3] so queries sit at columns
[0:512] and causality is data (two fixed triangle masks + per-core 0/1
column-kill scalars + exp bias columns); one uniform SPMD program serves all
cores.

All matmul operands are bf16 (fp32 PSUM accumulation). Activations are
feature-major ([feature, token]); V is projected token-major directly
(lhsT = x tile), so no on-device transposes at all. Scores are [k, q];
softmax needs no max subtraction (RMS-normalized q,k bound |s|/sqrt(D)).
Attention is causally trimmed: per head 8 N=512 score tiles (both chunks)
+ 8 N=256 tiles (far chunk only). Partition reductions (RMS ssq, softmax
den) are rank-1 matmuls; per-token broadcasts are one-hot-row matmuls;
reciprocals are batched and use the fast approx DVE op.
"""

import math
import numpy as np
from contextlib import ExitStack

import concourse.bass as bass
import concourse.mybir as mybir
import concourse.tile as tile
from concourse import bacc
from concourse.bass_utils import run_bass_kernel_spmd

F32 = mybir.dt.float32
F32R = mybir.dt.float32r
BF16 = mybir.dt.bfloat16
ADD = mybir.AluOpType.add
MULT = mybir.AluOpType.mult
EXP = mybir.ActivationFunctionType.Exp
SQRT = mybir.ActivationFunctionType.Sqrt
SQUARE = mybir.ActivationFunctionType.Square

EPS = 1e-8
NEG = -50.0  # additive pre-exp kill; exp(-50 + |s|max~11.4) ~ 2e-17


def full_cfg():
    return dict(B=2, S=2048, E=2048, D=128, G=2)


def derived(cfg):
    B, S, E, D, G = cfg["B"], cfg["S"], cfg["E"], cfg["D"], cfg["G"]
    NH = E // D            # 16 query heads
    ET = E // 128          # 16 contraction tiles
    NKT = S // 128         # 16 key tiles
    QPC = S // 4           # 512 query tokens per core
    CH = S // 8            # 256 chunk size
    TD = CH // 128         # 2 diagonal key-tiles per chunk
    GS = NH // G           # 8 heads per kv group
    return NH, ET, NKT, QPC, CH, TD, GS


def build_program(cfg, debug=False):
    B, S, E, D, G = cfg["B"], cfg["S"], cfg["E"], cfg["D"], cfg["G"]
    NH, ET, NKT, QPC, CH, TD, GS = derived(cfg)
    SCALE = 1.0 / math.sqrt(D)
    KC = 512
    NKC = S // KC          # 4 key-column chunks for K projection

    nc = bacc.Bacc()
    xT_d = nc.dram_tensor("xT", [E, S], BF16, kind="ExternalInput")
    wq_d = nc.dram_tensor("Wq", [E, E], BF16, kind="ExternalInput")
    wk_d = nc.dram_tensor("Wk", [E, G * D], BF16, kind="ExternalInput")
    wv_d = nc.dram_tensor("Wv", [E, G * D], BF16, kind="ExternalInput")
    wo_d = nc.dram_tensor("Wo", [E, E], BF16, kind="ExternalInput")
    bq_d = nc.dram_tensor("bq_t", [128, NH], F32, kind="ExternalInput")
    bk_d = nc.dram_tensor("bk_t", [128, G], F32, kind="ExternalInput")
    bv_d = nc.dram_tensor("bv_r", [1, G * D], BF16, kind="ExternalInput")
    bo_d = nc.dram_tensor("bo_t", [128, ET], F32, kind="ExternalInput")
    gq_d = nc.dram_tensor("gq_c", [128, 1], F32, kind="ExternalInput")
    gk_d = nc.dram_tensor("gk_c", [128, 1], F32, kind="ExternalInput")
    mask_d = nc.dram_tensor("mask2", [TD * 128, CH], BF16, kind="ExternalInput")
    oneh2_d = nc.dram_tensor("oneh2", [G, 256], BF16, kind="ExternalInput")
    oneh16_d = nc.dram_tensor("oneh16", [NH, NH * 128], BF16, kind="ExternalInput")
    ohT2f_d = nc.dram_tensor("ohT2f", [128, 2 * G], F32, kind="ExternalInput")
    ohT2b_d = nc.dram_tensor("ohT2b", [128, 2 * G], BF16, kind="ExternalInput")
    ohT16_d = nc.dram_tensor("ohT16", [128, NH * NH], F32, kind="ExternalInput")
    ones_d = nc.dram_tensor("ones1", [128, 1], F32, kind="ExternalInput")
    bcolB_d = nc.dram_tensor("bcolB", [128, 6], F32, kind="ExternalInput")
    akill_d = nc.dram_tensor("akill", [128, 6], F32, kind="ExternalInput")
    out_d = nc.dram_tensor("outT", [E, QPC], BF16, kind="ExternalOutput")
    if debug:
        dbg_pt_d = nc.dram_tensor("dbg_pt", [128, 3 * QPC], F32, kind="ExternalOutput")
        dbg_den_d = nc.dram_tensor("dbg_den", [1, QPC], F32, kind="ExternalOutput")
        dbg_cx_d = nc.dram_tensor("dbg_cx", [128, QPC], F32, kind="ExternalOutput")
        dbg_q_d = nc.dram_tensor("dbg_q", [128, QPC], F32, kind="ExternalOutput")
        dbg_k_d = nc.dram_tensor("dbg_k", [128, 2 * 128], F32, kind="ExternalOutput")
        dbg_ctx_d = nc.dram_tensor("dbg_ctx", [128, NH * QPC], F32, kind="ExternalOutput")
        dbg_b2_d = nc.dram_tensor("dbg_b2", [128, QPC], F32, kind="ExternalOutput")

    xT_r = xT_d.rearrange("(t p) s -> p t s", p=128)   # [128, ET, S]
    wq_r = wq_d.rearrange("(t p) c -> p t c", p=128)   # [128, ET, E]
    wk_r = wk_d.rearrange("(t p) c -> p t c", p=128)   # [128, ET, G*D]
    wv_r = wv_d.rearrange("(t p) c -> p t c", p=128)
    wo_r = wo_d.rearrange("(t p) c -> p t c", p=128)

    def r(ap):
        return ap if ap.dtype == F32R else ap.bitcast(F32R)

    with tile.TileContext(nc) as tc, ExitStack() as top:
        consts = top.enter_context(tc.tile_pool(name="consts", bufs=1))
        persist = top.enter_context(tc.tile_pool(name="persist", bufs=1))
        wkvp = top.enter_context(tc.tile_pool(name="wkv", bufs=1))

        # startup-critical DMAs first: x chunk 0 (fine-grained) and Wk/Wv
        xall = persist.tile([128, ET, S], BF16, tag="xall")
        for e4 in range(4):
            nc.sync.dma_start(
                out=xall[:, 4 * e4:4 * e4 + 4, 0:KC],
                in_=xT_r[:, 4 * e4:4 * e4 + 4, 0:KC])
        wk_sb = wkvp.tile([128, ET, G * D], BF16, tag="wk")
        for e4 in range(4):
            nc.sync.dma_start(out=wk_sb[:, 4 * e4:4 * e4 + 4, :],
                              in_=wk_r[:, 4 * e4:4 * e4 + 4, :])
        wv_sb = wkvp.tile([128, ET, G * D], BF16, tag="wv")
        nc.sync.dma_start(out=wv_sb, in_=wv_r)
        nc.sync.dma_start(out=xall[:, :, KC:2 * KC], in_=xT_r[:, :, KC:2 * KC])
        ones_col = consts.tile([128, 1], F32R)
        nc.sync.dma_start(out=ones_col, in_=ones_d[:, :].bitcast(F32R))
        ones_bf = consts.tile([128, 1], BF16)
        nc.vector.memset(ones_bf, 1.0)
        ones_rb = consts.tile([1, 128], BF16)
        nc.vector.memset(ones_rb, 1.0)
        ones_row = consts.tile([1, 128], BF16)
        nc.vector.memset(ones_row, 1.0)
        eps2 = consts.tile([G, 1], F32)
        nc.vector.memset(eps2, EPS)
        eps16 = consts.tile([NH, 1], F32)
        nc.vector.memset(eps16, EPS)
        gq_sb = consts.tile([128, 1], F32)
        nc.sync.dma_start(out=gq_sb, in_=gq_d[:, :])
        gk_sb = consts.tile([128, 1], F32)
        nc.sync.dma_start(out=gk_sb, in_=gk_d[:, :])
        bq_sb = consts.tile([128, NH], F32)
        nc.sync.dma_start(out=bq_sb, in_=bq_d[:, :])
        bk_sb = consts.tile([128, G], F32)
        nc.sync.dma_start(out=bk_sb, in_=bk_d[:, :])
        bv_sb = consts.tile([1, G * D], BF16)
        nc.sync.dma_start(out=bv_sb, in_=bv_d[:, :])
        bo_sb = consts.tile([128, ET], F32)
        nc.sync.dma_start(out=bo_sb, in_=bo_d[:, :])
        oneh2_sb = consts.tile([G, 256], BF16)
        nc.sync.dma_start(out=oneh2_sb, in_=oneh2_d[:, :])
        oneh16_sb = consts.tile([NH, NH * 128], BF16)
        nc.sync.dma_start(out=oneh16_sb, in_=oneh16_d[:, :])
        ohT2f_sb = consts.tile([128, 2 * G], F32R)
        nc.sync.dma_start(out=ohT2f_sb, in_=ohT2f_d[:, :].bitcast(F32R))
        ohT2b_sb = consts.tile([128, 2 * G], BF16)
        nc.sync.dma_start(out=ohT2b_sb, in_=ohT2b_d[:, :])
        ohT16_sb = consts.tile([128, NH * NH], F32R)
        nc.sync.dma_start(out=ohT16_sb, in_=ohT16_d[:, :].bitcast(F32R))
        bcolB_sb = consts.tile([128, 6], F32)
        nc.sync.dma_start(out=bcolB_sb, in_=bcolB_d[:, :])
        akill_sb = consts.tile([128, 6], F32)
        nc.sync.dma_start(out=akill_sb, in_=akill_d[:, :])
        mask_sb = []
        for t in range(TD):
            m = consts.tile([128, CH], BF16, tag=f"mask{t}", name=f"mask{t}")
            nc.sync.dma_start(out=m, in_=mask_d[t * 128:(t + 1) * 128, :])
            mask_sb.append(m)

        # bulk x (key chunks 2-3) after everything startup-critical
        for kc in range(2, NKC):
            nc.sync.dma_start(out=xall[:, :, kc * KC:(kc + 1) * KC],
                              in_=xT_r[:, :, kc * KC:(kc + 1) * KC])

        ktn = persist.tile([128, G, S], BF16, tag="ktn")
        vtok = persist.tile([128, NKT, G * D], BF16, tag="vtok")
        qtn = persist.tile([128, NH, QPC], BF16, tag="qtn")
        ctxt = persist.tile([128, NH, QPC], BF16, tag="ctxt")

        # ---------------- phase 1: K,V projections -----------------------
        with ExitStack() as p1:
            sqp = p1.enter_context(tc.tile_pool(name="sq1", bufs=3))
            smp = p1.enter_context(tc.tile_pool(name="sm1", bufs=2))
            pk = p1.enter_context(tc.tile_pool(name="pk", bufs=2, space="PSUM"))
            pv = p1.enter_context(tc.tile_pool(name="pv", bufs=2, space="PSUM"))
            pss = p1.enter_context(tc.tile_pool(name="pss", bufs=1, space="PSUM"))
            pbc = p1.enter_context(tc.tile_pool(name="pbc", bufs=2, space="PSUM"))

            # bv broadcast [128, G*D] in SBUF (outer product, done once)
            pbv = pbc.tile([128, G * D], F32, tag="pbc", name="pbv")
            nc.tensor.matmul(pbv, lhsT=ones_row, rhs=bv_sb,
                             start=True, stop=True)
            bvb_sb = wkvp.tile([128, G * D], F32, tag="bvb")
            nc.scalar.copy(out=bvb_sb, in_=pbv)

            pending = []

            def flush():
                while pending:
                    pending.pop(0)()

            for kc in range(NKC):
                ksl = slice(kc * KC, (kc + 1) * KC)
                # K projection, feature-major: out [128 (g*d), 512 tok]
                kaccs = []
                for g in range(G):
                    acc = pk.tile([128, KC], F32, tag="pk", name=f"kacc{g}")
                    for et in range(ET):
                        nc.tensor.matmul(
                            acc, lhsT=wk_sb[:, et, g * D:(g + 1) * D],
                            rhs=xall[:, et, ksl],
                            start=(et == 0), stop=(et == ET - 1))
                    kaccs.append(acc)
                # V projection, token-major: out [128 tok, G*D]
                vaccs = []
                for tb in range(4):
                    vacc = pv.tile([128, G * D], F32, tag="pv", name=f"vacc{tb}")
                    tsl = slice(kc * KC + tb * 128, kc * KC + (tb + 1) * 128)
                    for et in range(ET):
                        nc.tensor.matmul(
                            vacc, lhsT=xall[:, et, tsl], rhs=wv_sb[:, et, :],
                            start=(et == 0), stop=(et == ET - 1))
                    vaccs.append(vacc)

                def post_kv(kc=kc, ksl=ksl, kaccs=kaccs, vaccs=vaccs):
                    ss = pss.tile([G, KC], F32, tag="pss", name="ss")
                    for g in range(G):
                        # bias add, write raw K into ktn (bf16)
                        nc.vector.tensor_scalar(
                            out=ktn[:, g, ksl], in0=kaccs[g],
                            scalar1=bk_sb[:, g:g + 1], scalar2=None, op0=ADD)
                        sq = sqp.tile([128, KC], F32R, tag="sq", name="sq")
                        nc.scalar.activation(out=sq, in_=ktn[:, g, ksl],
                                             func=SQUARE)
                        nc.tensor.matmul(ss, lhsT=r(ohT2f_sb[:, 2 * g:2 * g + 2]),
                                         rhs=sq, start=(g == 0),
                                         stop=(g == G - 1))
                    srt = smp.tile([G, KC], F32, tag="srt", name="srt")
                    nc.scalar.activation(out=srt, in_=ss, func=SQRT,
                                         scale=1.0 / D, bias=eps2[:, :])
                    rk = smp.tile([G, KC], F32, tag="rk", name="rk")
                    nc.vector.reciprocal_approx_fast(out=rk, in_=srt)
                    rkb = smp.tile([G, KC], BF16, tag="rkb", name="rkb")
                    nc.vector.tensor_copy(out=rkb, in_=rk)
                    for g in range(G):
                        bc = pbc.tile([128, KC], F32, tag="pbc", name="bc")
                        nc.tensor.matmul(
                            bc, lhsT=oneh2_sb[:, g * 128:(g + 1) * 128],
                            rhs=rkb, start=True, stop=True)
                        # ktn = (ktn_raw * gk) * (1/rms)   in place
                        nc.vector.scalar_tensor_tensor(
                            out=ktn[:, g, ksl], in0=ktn[:, g, ksl],
                            scalar=gk_sb[:, :], in1=bc, op0=MULT, op1=MULT)
                    for tb in range(4):
                        kt = kc * 4 + tb
                        nc.vector.tensor_tensor(
                            out=vtok[:, kt, :], in0=vaccs[tb], in1=bvb_sb,
                            op=ADD)
                pending.append(post_kv)
                if kc >= 1:
                    pending.pop(0)()
            flush()

        # ---------------- phase 2: Q projection ---------------------------
        with ExitStack() as p2:
            wqp = p2.enter_context(tc.tile_pool(name="wqs", bufs=3))
            sqp2 = p2.enter_context(tc.tile_pool(name="sq2", bufs=3))
            smp2 = p2.enter_context(tc.tile_pool(name="sm2", bufs=1))
            pq = p2.enter_context(tc.tile_pool(name="pq", bufs=2, space="PSUM"))
            psq = p2.enter_context(tc.tile_pool(name="psq", bufs=1, space="PSUM"))
            pbc2p = p2.enter_context(tc.tile_pool(name="pbcq", bufs=2, space="PSUM"))

            ssq_all = psq.tile([NH, QPC], F32, tag="ssqa")
            pending2 = []
            wq_tiles = []
            for qc in range(min(2, NH)):
                w = wqp.tile([128, ET, 128], BF16, tag="wq", name="wq")
                nc.sync.dma_start(out=w,
                                  in_=wq_r[:, :, qc * 128:(qc + 1) * 128])
                wq_tiles.append(w)
            for qc in range(NH):
                if qc + 2 < NH:
                    w = wqp.tile([128, ET, 128], BF16, tag="wq", name="wq")
                    nc.sync.dma_start(
                        out=w, in_=wq_r[:, :, (qc + 2) * 128:(qc + 3) * 128])
                    wq_tiles.append(w)
                wq_sb = wq_tiles[qc]
                acc = pq.tile([128, QPC], F32, tag="pq", name="qacc")
                for et in range(ET):
                    nc.tensor.matmul(acc, lhsT=wq_sb[:, et, :],
                                     rhs=xall[:, et, 0:QPC],
                                     start=(et == 0), stop=(et == ET - 1))

                def post_q(qc=qc, acc=acc):
                    nc.vector.tensor_scalar(
                        out=qtn[:, qc, :], in0=acc,
                        scalar1=bq_sb[:, qc:qc + 1], scalar2=None, op0=ADD)
                    sq = sqp2.tile([128, QPC], F32R, tag="sq", name="qsq")
                    nc.scalar.activation(out=sq, in_=qtn[:, qc, :], func=SQUARE)
                    nc.tensor.matmul(
                        ssq_all, lhsT=r(ohT16_sb[:, qc * NH:(qc + 1) * NH]),
                        rhs=sq, start=(qc == 0), stop=(qc == NH - 1))
                pending2.append(post_q)
                if qc >= 1:
                    pending2.pop(0)()
            while pending2:
                pending2.pop(0)()

            srtq = smp2.tile([NH, QPC], F32, tag="srtq")
            nc.scalar.activation(out=srtq, in_=ssq_all, func=SQRT,
                                 scale=1.0 / D, bias=eps16[:, :])
            rq = smp2.tile([NH, QPC], F32, tag="rq")
            nc.vector.reciprocal_approx_fast(out=rq, in_=srtq)
            rqb = smp2.tile([NH, QPC], BF16, tag="rqb")
            nc.vector.tensor_copy(out=rqb, in_=rq)
            for qc in range(NH):
                bc = pbc2p.tile([128, QPC], F32, tag="pbcq", name="qbc")
                nc.tensor.matmul(
                    bc, lhsT=oneh16_sb[:, qc * 128:(qc + 1) * 128],
                    rhs=rqb, start=True, stop=True)
                nc.vector.scalar_tensor_tensor(
                    out=qtn[:, qc, :], in0=qtn[:, qc, :], scalar=gq_sb[:, :],
                    in1=bc, op0=MULT, op1=MULT)

        if debug:
            with tc.tile_pool(name="dbgp", bufs=1) as dbgp:
                tq = dbgp.tile([128, QPC], F32, tag="tq")
                nc.scalar.copy(out=tq, in_=qtn[:, 0, :])
                nc.sync.dma_start(out=dbg_q_d[:, :], in_=tq)
                tk = dbgp.tile([128, 256], F32, tag="tk")
                nc.scalar.copy(out=tk, in_=ktn[:, 0, 0:256])
                nc.sync.dma_start(out=dbg_k_d[:, :], in_=tk)

        # ---------------- phase 3: attention ------------------------------
        # per head: tiles 0,1 (N=512, A triangle-masked in place, B full),
        #           tiles 2,3 (N=256 B-only, triangle),
        #           tiles 4..9 (N=512, A kill-scalars), 10..15 (N=256, bias).
        # pt is always full [128,512] (A-half zeroed on narrow tiles) so den
        # and cx are single clean accumulation chains at N=512.
        with ExitStack() as p3:
            ptp = p3.enter_context(tc.tile_pool(name="pt", bufs=5))
            rdp = p3.enter_context(tc.tile_pool(name="rdp", bufs=2))
            psc = p3.enter_context(tc.tile_pool(name="psc", bufs=3, space="PSUM"))
            pden = p3.enter_context(tc.tile_pool(name="pden", bufs=2, space="PSUM"))
            pcx = p3.enter_context(tc.tile_pool(name="pcx", bufs=3, space="PSUM"))

            pend3 = []

            def flush3(lag=0):
                while len(pend3) > lag:
                    pend3.pop(0)()

            N512 = (0, 1, 4, 5, 6, 7, 8, 9)
            for hp in range(NH // 2):
                h0 = 2 * hp
                g = h0 // GS
                den = pden.tile([G, QPC], F32, tag="den", name="den")
                cxs_ = {}
                for h in (h0, h0 + 1):
                    cxs_[h] = pcx.tile([128, QPC], F32, tag="cx", name="cx")
                for t in range(NKT):
                    wide = t in N512
                    ksl = slice(t * 128, (t + 1) * 128)
                    for hi, h in enumerate((h0, h0 + 1)):
                        if wide:
                            sc = psc.tile([128, QPC], F32, tag="sc",
                                          name="sc")
                            nc.tensor.matmul(sc, lhsT=ktn[:, g, ksl],
                                             rhs=qtn[:, h, :], start=True,
                                             stop=True)
                        else:
                            sc = psc.tile([128, QPC], F32, tag="sc",
                                          name="scn")
                            nc.tensor.matmul(sc[:, 0:CH],
                                             lhsT=ktn[:, g, ksl],
                                             rhs=qtn[:, h, CH:QPC],
                                             start=True, stop=True)

                        def post_t(h=h, hi=hi, g=g, t=t, wide=wide, sc=sc,
                                   den=den, cx=cxs_[h]):
                            if wide:
                                pt = ptp.tile([128, QPC], BF16, tag="pt",
                                              name="pt")
                                nc.scalar.activation(out=pt, in_=sc,
                                                     func=EXP, scale=SCALE)
                                if t < 2:
                                    nc.vector.tensor_tensor(
                                        out=pt[:, 0:CH], in0=pt[:, 0:CH],
                                        in1=mask_sb[t], op=MULT)
                                else:
                                    nc.vector.tensor_scalar(
                                        out=pt[:, 0:CH], in0=pt[:, 0:CH],
                                        scalar1=akill_sb[:, t - 4:t - 3],
                                        scalar2=None, op0=MULT)
                            else:
                                pt = ptp.tile([128, QPC], BF16, tag="pt",
                                              name="ptn")
                                nc.gpsimd.memset(pt[:, 0:CH], 0.0)
                                if t >= 10:
                                    nc.scalar.activation(
                                        out=pt[:, CH:QPC], in_=sc[:, 0:CH],
                                        func=EXP, scale=SCALE,
                                        bias=bcolB_sb[:, t - 10:t - 9])
                                else:
                                    nc.scalar.activation(
                                        out=pt[:, CH:QPC], in_=sc[:, 0:CH],
                                        func=EXP, scale=SCALE)
                                    nc.vector.tensor_tensor(
                                        out=pt[:, CH:QPC],
                                        in0=pt[:, CH:QPC],
                                        in1=mask_sb[t - 2], op=MULT)
                            nc.tensor.matmul(
                                den, lhsT=ohT2b_sb[:, 2 * hi:2 * hi + 2],
                                rhs=pt, start=(t == 0 and hi == 0),
                                stop=(t == NKT - 1 and hi == 1),
                                skip_group_check=True)
                            nc.tensor.matmul(
                                cx, lhsT=vtok[:, t, g * D:(g + 1) * D],
                                rhs=pt, start=(t == 0),
                                stop=(t == NKT - 1))
                        pend3.append(post_t)
                        flush3(lag=3)

                def post_pair(h0=h0, den=den, cxs_=cxs_):
                    rd = rdp.tile([G, QPC], F32, tag="rd", name="rd")
                    nc.vector.reciprocal_approx_fast(out=rd, in_=den)
                    rdb = rdp.tile([G, QPC], BF16, tag="rdb", name="rdb")
                    nc.vector.tensor_copy(out=rdb, in_=rd)
                    for hi, h in enumerate((h0, h0 + 1)):
                        b2 = psc.tile([128, QPC], F32, tag="sc", name="b2")
                        nc.tensor.matmul(
                            b2, lhsT=oneh2_sb[:, hi * 128:(hi + 1) * 128],
                            rhs=rdb, start=True, stop=True)
                        cxsb = rdp.tile([128, QPC], BF16, tag="cxs",
                                        name="cxs")
                        nc.scalar.copy(out=cxsb, in_=cxs_[h])
                        nc.vector.tensor_tensor(out=ctxt[:, h, :], in0=cxsb,
                                                in1=b2, op=MULT)
                pend3.append(post_pair)
            flush3()

        # ---------------- phase 4: output projection ----------------------
        with ExitStack() as p4:
            wop = p4.enter_context(tc.tile_pool(name="wos", bufs=3))
            osb = p4.enter_context(tc.tile_pool(name="osb", bufs=3))
            po = p4.enter_context(tc.tile_pool(name="po", bufs=3, space="PSUM"))
            pend4 = []
            wo_tiles = []
            for c2 in range(min(2, ET)):
                w = wop.tile([128, ET, 128], BF16, tag="wo", name="wo")
                nc.sync.dma_start(out=w,
                                  in_=wo_r[:, :, c2 * 128:(c2 + 1) * 128])
                wo_tiles.append(w)
            for c2 in range(ET):
                if c2 + 2 < ET:
                    w = wop.tile([128, ET, 128], BF16, tag="wo", name="wo")
                    nc.sync.dma_start(
                        out=w, in_=wo_r[:, :, (c2 + 2) * 128:(c2 + 3) * 128])
                    wo_tiles.append(w)
                wo_sb = wo_tiles[c2]
                acc = po.tile([128, QPC], F32, tag="po", name="oacc")
                for ct in range(ET):
                    nc.tensor.matmul(acc, lhsT=wo_sb[:, ct, :],
                                     rhs=ctxt[:, ct, :],
                                     start=(ct == 0), stop=(ct == ET - 1))

                def post_o(c2=c2, acc=acc):
                    ot = osb.tile([128, QPC], BF16, tag="ot", name="ot")
                    nc.vector.tensor_scalar(
                        out=ot, in0=acc, scalar1=bo_sb[:, c2:c2 + 1],
                        scalar2=None, op0=ADD)
                    nc.sync.dma_start(
                        out=out_d[c2 * 128:(c2 + 1) * 128, :], in_=ot)
                pend4.append(post_o)
                if c2 >= 1:
                    pend4.pop(0)()
            while pend4:
                pend4.pop(0)()
    nc.compile()
    return nc


# ---------------------------------------------------------------------------
# host-side sharding
# ---------------------------------------------------------------------------

def core_perm(cfg, j):
    """Permutation of token positions for quarter j: [A | B | c1 | c2 | c3]."""
    S = cfg["S"]
    CH = S // 8
    A = np.arange(CH * j, CH * (j + 1))
    Bc = np.arange(S - CH * (j + 1), S - CH * j)
    rest = np.setdiff1d(np.arange(S), np.concatenate([A, Bc]))
    c1 = rest[rest < CH * j]
    c3 = rest[rest >= S - CH * j]
    c2 = rest[(rest >= CH * j) & (rest < S - CH * j)]
    perm = np.concatenate([A, Bc, c1, c2, c3])
    assert perm.shape == (S,)
    return perm


def tri_masks(cfg, dtype):
    S = cfg["S"]
    CH = S // 8
    TD = CH // 128
    m = np.zeros((TD * 128, CH), np.float32)
    for t in range(TD):
        kk = np.arange(128)[:, None] + t * 128
        qq = np.arange(CH)[None, :]
        m[t * 128:(t + 1) * 128, :] = (kk <= qq).astype(np.float32)
    return m.astype(dtype)


def make_in_maps(cfg, inputs):
    import ml_dtypes
    bf16 = ml_dtypes.bfloat16
    B, S, E, D, G = cfg["B"], cfg["S"], cfg["E"], cfg["D"], cfg["G"]
    NH, ET, NKT, QPC, CH, TD, GS = derived(cfg)
    x = np.asarray(inputs["x"], np.float32)

    oneh2 = np.zeros((G, 256), np.float32)
    oneh2[0, 0:128] = 1.0
    oneh2[1, 128:256] = 1.0
    oneh16 = np.zeros((NH, NH * 128), np.float32)
    for h in range(NH):
        oneh16[h, h * 128:(h + 1) * 128] = 1.0
    # stationary one-hot columns: slice [:, r*M:(r+1)*M] has ones in col r,
    # landing a rank-1 reduction on row r of an M-row PSUM tile
    ohT2 = np.zeros((128, 2 * G), np.float32)
    ohT2[:, 0] = 1.0
    ohT2[:, 3] = 1.0
    ohT16 = np.zeros((128, NH * NH), np.float32)
    for qc in range(NH):
        ohT16[:, qc * NH + qc] = 1.0

    shared = dict(
        Wq=np.ascontiguousarray(inputs["Wq"]).astype(bf16),
        Wk=np.ascontiguousarray(inputs["Wk"]).astype(bf16),
        Wv=np.ascontiguousarray(inputs["Wv"]).astype(bf16),
        Wo=np.ascontiguousarray(inputs["Wo"]).astype(bf16),
        bq_t=np.ascontiguousarray(
            np.asarray(inputs["bq"], np.float32).reshape(NH, 128).T),
        bk_t=np.ascontiguousarray(
            np.asarray(inputs["bk"], np.float32).reshape(G, 128).T),
        bv_r=np.asarray(inputs["bv"], np.float32).reshape(1, G * D).astype(bf16),
        bo_t=np.ascontiguousarray(
            np.asarray(inputs["bo"], np.float32).reshape(ET, 128).T),
        gq_c=np.asarray(inputs["gamma_q"], np.float32).reshape(128, 1).copy(),
        gk_c=np.asarray(inputs["gamma_k"], np.float32).reshape(128, 1).copy(),
        mask2=tri_masks(cfg, bf16),
        oneh2=oneh2.astype(bf16),
        oneh16=oneh16.astype(bf16),
        ohT2f=ohT2,
        ohT2b=ohT2.astype(bf16),
        ohT16=ohT16,
        ones1=np.ones((128, 1), np.float32),
    )
    in_maps, perms = [], []
    for c in range(8):
        b, j = c // 4, c % 4
        perm = core_perm(cfg, j)
        xt = np.ascontiguousarray(x[b].T[:, perm]).astype(bf16)
        # tiles 4..9 hold c1 (tokens before A): tile 4+i valid for A iff i < 2j
        ak = np.zeros((128, 6), np.float32)
        ak[:, 0:2 * j] = 1.0
        # tiles 10..15: valid for B iff t < 16-2j  (c2 vs c3)
        bc = np.full((128, 6), NEG, np.float32)
        bc[:, 0:6 - 2 * j] = 0.0
        m = dict(shared)
        m["xT"] = xt
        m["akill"] = ak
        m["bcolB"] = bc
        in_maps.append(m)
        perms.append(perm)
    return in_maps, perms


def assemble(cfg, results, perms):
    B, S, E = cfg["B"], cfg["S"], cfg["E"]
    QPC = S // 4
    out = np.empty((B, S, E), np.float32)
    for c in range(8):
        b = c // 4
        out[b, perms[c][:QPC], :] = np.asarray(results[c]["outT"], np.float32).T
    return out


_CACHE = {}


def kernel(**inputs):
    cfg = full_cfg()
    if "nc" not in _CACHE:
        _CACHE["nc"] = build_program(cfg)
    nc = _CACHE["nc"]
    in_maps, perms = make_in_maps(cfg, inputs)
    res = run_bass_kernel_spmd(nc, in_maps, list(range(8)))
    return assemble(cfg, res.results, perms)
